# revision 7
# baseline (speedup 1.0000x reference)
"""Trainium2 Bass kernel for nn_DCT: YCbCr 3x3 channel mix + 8x8 block DCT
(stride 8) + repeated min/max normalization collapsed to a per-channel affine.

Sharding: pure data parallel, batch 32 -> 4 samples on each of 8 NeuronCores.

v2 dataflow (per core, per sample s, per band r of 128 rows, f32 PSUM):
  - x band loaded as [96=(b4,ci,i), (g, w=512)] SBUF tile: partition stacks
    4 block-rows x 3 channels x 8 rows so the YCbCr mix rides M1's
    contraction; g indexes the 4 block-row groups of the band.
  - M1 (per chunk c, group g): lhsT = x_band[:, g, c*128:(c+1)*128] [96,128],
    rhs = BDCOLMIX [96, 96=(co,u,b4)] = y[co,ci]*D[u,i]*delta(b4).
    out T1m[:, co, g, u, b4] = colDCT of the YCbCr image (mix done in-k).
  - ACT copy T1m PSUM -> t1s SBUF bf16.
  - M2 (per chunk): ONE matmul lhsT = BDROW [128=(bw,j), 128=(v,bw)] const,
    rhs = t1s [128, 384=(co,g,u,b4)] -> T2[(v,bw), (co,g,u,b4)] = 2D DCT,
    then ONE rank-8 accumulating matmul lhsT = E8 [8,128]=delta(v=k),
    rhs = BRHS[s] [8, 384] = bias/scale adds b' = b/s into the same PSUM.
  - DVE: single fused pass out = T2 * s_tile (PSUM f32 x bf16 -> bf16 SBUF),
    per chunk-pair [128, 768]; this is the only vector work.
  - DMA out per (s, r): [128, 1536] contiguous bf16; host untangles.
"""

import math
import sys

import numpy as np

for _p in ("/opt/trn_rl_repo", "/opt/pypackages"):
    if _p not in sys.path:
        sys.path.insert(0, _p)

N = 8
IN_CH = 3
EPS = 1e-6
B_FULL = 32
H = 512
W = 512
NCORES = 8
BPC = B_FULL // NCORES  # samples per core
NBANDS = 4  # 512 rows / 128
NCHUNKS = 4  # 512 cols / 128

_CACHED_NC = None


def _dct_basis(n=N):
    u = np.arange(n)
    i = np.arange(n)
    b = np.cos(np.pi * np.outer(u, i + 0.5) / n)
    c = np.full(n, math.sqrt(2.0 / n))
    c[0] = math.sqrt(1.0 / n)
    return b * c[:, None]


def _build_const_tiles(ycbcr_w):
    """BDCOLMIX [96,96], BDROW [128,128], E8 [8,128] f64->f32 matmul consts."""
    D = _dct_basis()  # D[u, i]
    y = np.asarray(ycbcr_w, np.float64)  # [co, ci]
    bdcolmix = np.zeros((96, 96), np.float64)
    for b4 in range(4):
        for ci in range(3):
            for co in range(3):
                # rows (b4,ci,i) = b4*24+ci*8+i ; cols (co,u,b4') = co*32+u*4+b4
                rows = b4 * 24 + ci * 8 + np.arange(8)
                cols = co * 32 + np.arange(8) * 4 + b4
                bdcolmix[np.ix_(rows, cols)] = D.T * y[co, ci]
    bdrow = np.zeros((128, 128), np.float64)
    for bw in range(16):
        # rows (bw,j) = bw*8+j ; cols (v,bw') = v*16+bw ; value D[v,j]
        bdrow[bw * 8 : (bw + 1) * 8, np.arange(8) * 16 + bw] = D.T
    e8 = np.zeros((8, 128), np.float64)
    for v in range(8):
        e8[v, v * 16 : (v + 1) * 16] = 1.0
    return (
        bdcolmix.astype(np.float32),
        bdrow.astype(np.float32),
        e8.astype(np.float32),
    )


def _affine_coeffs(max_, min_):
    """Closed form of t -> (t - min)/d applied B_FULL times: out = s*dct + b."""
    m = np.asarray(max_, np.float32)[..., 0, 0]
    n = np.asarray(min_, np.float32)[..., 0, 0]
    d = (m - n + np.float32(EPS)).astype(np.float64)
    r = 1.0 / d
    s = r**B_FULL
    b = -n.astype(np.float64) * (r * (1.0 - s) / (1.0 - r))
    return s, b  # [B, 192] f64


def _build_nc():
    import concourse.mybir as mybir
    import concourse.tile as tile
    from concourse import bacc
    from contextlib import ExitStack

    f32 = mybir.dt.float32
    bf16 = mybir.dt.bfloat16
    nc = bacc.Bacc()
    # host pre-permuted: x[s, r, (b,ci,i)=96, g=4, w=512]
    x_t = nc.declare_dram_parameter("x", [BPC, NBANDS, 96, 4, W], bf16, isOutput=False)
    s_t = nc.declare_dram_parameter("s_tile", [BPC, 128, 2, 384], bf16, isOutput=False)
    brhs_t = nc.declare_dram_parameter("brhs", [BPC, 8, 384], bf16, isOutput=False)
    bdcolmix_t = nc.declare_dram_parameter("bdcolmix", [96, 96], bf16, isOutput=False)
    bdrow_t = nc.declare_dram_parameter("bdrow", [128, 128], bf16, isOutput=False)
    e8_t = nc.declare_dram_parameter("e8", [8, 128], bf16, isOutput=False)
    # Permuted output layout: [s, r, p=(v,bw), (c, co, g, u, b4)] bf16;
    # host untangles at gather time.
    out_t = nc.declare_dram_parameter("out", [BPC, NBANDS, 128, 1536], bf16, isOutput=True)

    x_v = x_t[:].rearrange("s r p g w -> s r p (g w)")

    with ExitStack() as ctx:
        tc = ctx.enter_context(tile.TileContext(nc))
        consts = ctx.enter_context(tc.tile_pool(name="consts", bufs=1))
        sbp = ctx.enter_context(tc.tile_pool(name="sbp", bufs=1))
        xp = ctx.enter_context(tc.tile_pool(name="xp", bufs=3))
        t1sb = ctx.enter_context(tc.tile_pool(name="t1sb", bufs=6))
        t1ps = ctx.enter_context(tc.tile_pool(name="t1ps", bufs=3, space="PSUM"))
        t2ps = ctx.enter_context(tc.tile_pool(name="t2ps", bufs=2, space="PSUM"))
        outp = ctx.enter_context(tc.tile_pool(name="outp", bufs=3))

        bdcolmix = consts.tile([96, 96], bf16)
        nc.gpsimd.dma_start(out=bdcolmix, in_=bdcolmix_t[:])
        bdrow = consts.tile([128, 128], bf16)
        nc.gpsimd.dma_start(out=bdrow, in_=bdrow_t[:])
        e8 = consts.tile([8, 128], bf16)
        nc.gpsimd.dma_start(out=e8, in_=e8_t[:])

        # per-sample affine operands (persist whole kernel; tiny)
        s_sb = sbp.tile([128, BPC, 2, 384], bf16)
        nc.gpsimd.dma_start(out=s_sb, in_=s_t[:].rearrange("s p q n -> p s q n"))
        brhs_sb = sbp.tile([8, BPC, 384], bf16)
        nc.gpsimd.dma_start(out=brhs_sb, in_=brhs_t[:].rearrange("s k n -> k s n"))

        for s in range(BPC):
            for r in range(NBANDS):
                x_band = xp.tile([96, NBANDS, W], bf16)  # f = (g, w)
                nc.sync.dma_start(out=x_band, in_=x_v[s, r])
                out_band = outp.tile([128, NCHUNKS, 384], bf16)  # f=(c,co,g,u,b4)
                for cp in range(2):
                    t2 = t2ps.tile([128, 2, 512], f32)  # 2 banks; use [:, ch, 0:384]
                    for ch in range(2):
                        c = cp * 2 + ch
                        t1m = t1ps.tile([128, 512], f32)  # use 0:384 = (co,g,u,b4)
                        t1v = t1m[:, 0:384].rearrange("p (co g ub) -> p co g ub", co=3, g=4)
                        for g in range(4):
                            nc.tensor.matmul(
                                t1v[:, :, g],
                                lhsT=x_band[:, g, c * 128 : (c + 1) * 128],
                                rhs=bdcolmix,
                                start=True,
                                stop=True,
                            )
                        t1s = t1sb.tile([128, 384], bf16)
                        nc.scalar.copy(out=t1s, in_=t1m[:, 0:384])
                        nc.tensor.matmul(
                            t2[:, ch, 0:384],
                            lhsT=bdrow,
                            rhs=t1s,
                            start=True,
                            stop=False,
                        )
                        nc.tensor.matmul(
                            t2[:, ch, 0:384],
                            lhsT=e8,
                            rhs=brhs_sb[:, s],
                            start=False,
                            stop=True,
                        )
                    # fused affine: out = (dct + b/s) * s ; single DVE pass
                    nc.vector.tensor_mul(
                        out=out_band[:, cp * 2 : (cp + 1) * 2],
                        in0=t2[:, :, 0:384],
                        in1=s_sb[:, s],
                    )
                nc.scalar.dma_start(
                    out=out_t[s, r],
                    in_=out_band.rearrange("p c n -> p (c n)"),
                )
    return nc


def _get_nc():
    global _CACHED_NC
    if _CACHED_NC is None:
        nc = _build_nc()
        if not nc.is_finalized():
            nc.finalize()
        _CACHED_NC = nc
    return _CACHED_NC


def _make_in_maps(x, max_, min_, ycbcr_w):
    import ml_dtypes

    bf16 = ml_dtypes.bfloat16
    x16 = np.asarray(x, np.float32).astype(bf16)
    # device layout: x[s, r, (b,ci,i), g, w] with h = r*128 + g*32 + b*8 + i
    xp = x16.reshape(-1, 3, 4, 4, 4, 8, W)  # s, ci, r, g, b, i, w
    xp = np.ascontiguousarray(xp.transpose(0, 2, 4, 1, 5, 3, 6))  # s,r,b,ci,i,g,w
    xp = xp.reshape(-1, NBANDS, 96, 4, W)
    s, b = _affine_coeffs(max_, min_)  # [32, 192] f64
    bp = b / s  # pre-divided bias: device computes (dct + b/s) * s
    # [B, co, u, v] -> tiles
    s4 = s.reshape(-1, 3, 8, 8)
    bp4 = bp.reshape(-1, 3, 8, 8)
    # s_tile [B, p=(v,bw)=128, q=2, n=(co,g,u,b4)=384]
    s_tile = np.zeros((s4.shape[0], 8, 16, 2, 3, 4, 8, 4), np.float64)
    s_tile[:] = s4.transpose(0, 3, 1, 2)[:, :, None, None, :, None, :, None]
    s_tile = s_tile.reshape(-1, 128, 2, 384)
    # brhs [B, 8=v, n=(co,g,u,b4)=384]
    brhs = np.zeros((s4.shape[0], 8, 3, 4, 8, 4), np.float64)
    brhs[:] = bp4.transpose(0, 3, 1, 2)[:, :, :, None, :, None]
    brhs = brhs.reshape(-1, 8, 384)

    bdcolmix, bdrow, e8 = _build_const_tiles(np.asarray(ycbcr_w, np.float32))

    in_maps = []
    for core in range(NCORES):
        sl = slice(core * BPC, (core + 1) * BPC)
        in_maps.append(
            {
                "x": np.ascontiguousarray(xp[sl]),
                "s_tile": s_tile[sl].astype(bf16),
                "brhs": brhs[sl].astype(bf16),
                "bdcolmix": bdcolmix.astype(bf16),
                "bdrow": bdrow.astype(bf16),
                "e8": e8.astype(bf16),
            }
        )
    return in_maps


def kernel(x, max_, min_, ycbcr_w, dct_w):
    from concourse.bass_utils import run_bass_kernel_spmd

    nc = _get_nc()
    in_maps = _make_in_maps(x, max_, min_, ycbcr_w)
    res = run_bass_kernel_spmd(nc, in_maps, core_ids=list(range(NCORES)))
    out = np.concatenate([res.results[i]["out"] for i in range(NCORES)], axis=0)
    return _untangle(out)


def _untangle(dev_out):
    """[B, r=4, p=128, f=1536] device layout -> [B, 192, 64, 64] f32."""
    v = np.asarray(dev_out).astype(np.float32)
    # p = (v=8, bw=16) ; f = (c=4, co=3, g=4, u=8, b4=4)
    v = v.reshape(-1, 4, 8, 16, 4, 3, 4, 8, 4)  # s, r, v, bw, c, co, g, u, b4
    v = v.transpose(0, 5, 7, 2, 1, 6, 8, 4, 3)  # s, co, u, v, r, g, b4, c, bw
    return np.ascontiguousarray(v.reshape(-1, 192, 64, 64))


# revision 8
# speedup vs baseline: 1.4516x; 1.4516x over previous
"""Trainium2 Bass kernel for nn_DCT: YCbCr 3x3 channel mix + 8x8 block DCT
(stride 8) + repeated min/max normalization collapsed to a per-channel affine.

Sharding: pure data parallel, batch 32 -> 4 samples on each of 8 NeuronCores.

v3 dataflow — direct 2D DCT, everything inside one matmul stage:
  - Host shuffles x to block-pixel layout: xa[s, (ci01,i,j)=128, blk=4096],
    xb[s, (ci2,i,j)+ones=65, blk=4096] (row 64 = 1.0; blk = hb*64+wb).
  - Per-sample rhs constants carry mix, both DCTs, the affine scale AND bias:
      K[(ci,i,j),(co,u,v)] = y[co,ci]*D[u,i]*D[v,j]*s_aff[smp,co,u,v]
      K1 = K rows 0:128, K2 = K rows 128:192 + row 64 = b_aff[smp].
  - Per (s, group of 128 blocks): two accumulating matmuls
      ps[blk128, (co,u,v)] = xa_g^T @ K1 + xb_g^T @ K2   (f32 PSUM)
    give the FINAL normalized DCT output directly.
  - PSUM -> SBUF bf16 copies split between Scalar and Vector engines.
  - One contiguous 1.5MB DMA out per sample; host untangles.
"""

import math
import sys

import numpy as np

for _p in ("/opt/trn_rl_repo", "/opt/pypackages"):
    if _p not in sys.path:
        sys.path.insert(0, _p)

N = 8
IN_CH = 3
EPS = 1e-6
B_FULL = 32
H = 512
W = 512
NCORES = 8
BPC = B_FULL // NCORES  # samples per core
NBLK = 4096  # 64x64 blocks per image
NGRP = NBLK // 128  # 32 groups of 128 blocks

_CACHED_NC = None


def _dct_basis(n=N):
    u = np.arange(n)
    i = np.arange(n)
    b = np.cos(np.pi * np.outer(u, i + 0.5) / n)
    c = np.full(n, math.sqrt(2.0 / n))
    c[0] = math.sqrt(1.0 / n)
    return b * c[:, None]


def _affine_coeffs(max_, min_):
    """Closed form of t -> (t - min)/d applied B_FULL times: out = s*dct + b."""
    m = np.asarray(max_, np.float32)[..., 0, 0]
    n = np.asarray(min_, np.float32)[..., 0, 0]
    d = (m - n + np.float32(EPS)).astype(np.float64)
    r = 1.0 / d
    s = r**B_FULL
    b = -n.astype(np.float64) * (r * (1.0 - s) / (1.0 - r))
    return s, b  # [B, 192] f64


def _build_nc():
    import concourse.mybir as mybir
    import concourse.tile as tile
    from concourse import bacc
    from contextlib import ExitStack

    f32 = mybir.dt.float32
    bf16 = mybir.dt.bfloat16
    nc = bacc.Bacc()
    xa_t = nc.declare_dram_parameter("xa", [BPC, 128, NBLK], bf16, isOutput=False)
    xb_t = nc.declare_dram_parameter("xb", [BPC, 65, NBLK], bf16, isOutput=False)
    k1_t = nc.declare_dram_parameter("k1", [BPC, 128, 192], bf16, isOutput=False)
    k2_t = nc.declare_dram_parameter("k2", [BPC, 65, 192], bf16, isOutput=False)
    # out[s, p=blk%128, g=blk//128, (co,u,v)]
    out_t = nc.declare_dram_parameter("out", [BPC, 128, NGRP, 192], bf16, isOutput=True)

    with ExitStack() as ctx:
        tc = ctx.enter_context(tile.TileContext(nc))
        consts = ctx.enter_context(tc.tile_pool(name="consts", bufs=1))
        xap = ctx.enter_context(tc.tile_pool(name="xap", bufs=2))
        xbp = ctx.enter_context(tc.tile_pool(name="xbp", bufs=2))
        psp = ctx.enter_context(tc.tile_pool(name="psp", bufs=3, space="PSUM"))
        outp = ctx.enter_context(tc.tile_pool(name="outp", bufs=2))

        k1_sb = consts.tile([128, BPC, 192], bf16)
        nc.gpsimd.dma_start(out=k1_sb, in_=k1_t[:].rearrange("s k n -> k s n"))
        k2_sb = consts.tile([65, BPC, 192], bf16)
        nc.gpsimd.dma_start(out=k2_sb, in_=k2_t[:].rearrange("s k n -> k s n"))

        for s in range(BPC):
            xa = xap.tile([128, NBLK], bf16)
            for q in range(4):
                nc.sync.dma_start(
                    out=xa[:, q * 1024 : (q + 1) * 1024],
                    in_=xa_t[s][:, q * 1024 : (q + 1) * 1024],
                )
            xb = xbp.tile([65, NBLK], bf16)
            for q in range(2):
                nc.gpsimd.dma_start(
                    out=xb[:, q * 2048 : (q + 1) * 2048],
                    in_=xb_t[s][:, q * 2048 : (q + 1) * 2048],
                )
            out_sb = outp.tile([128, NGRP, 192], bf16)
            for t4 in range(NGRP // 4):  # 8 PSUM tiles x 4 groups
                ps = psp.tile([128, 4, 256], f32)  # 2 banks; use [:, q, 0:192]
                for q in range(4):
                    g = t4 * 4 + q
                    nc.tensor.matmul(
                        ps[:, q, 0:192],
                        lhsT=xa[:, g * 128 : (g + 1) * 128],
                        rhs=k1_sb[:, s],
                        start=True,
                        stop=False,
                    )
                    nc.tensor.matmul(
                        ps[:, q, 0:192],
                        lhsT=xb[:, g * 128 : (g + 1) * 128],
                        rhs=k2_sb[:, s],
                        start=False,
                        stop=True,
                    )
                dst = out_sb[:, t4 * 4 : (t4 + 1) * 4]
                if t4 % 2 == 0:
                    nc.scalar.copy(out=dst, in_=ps[:, :, 0:192])
                else:
                    nc.vector.tensor_copy(out=dst, in_=ps[:, :, 0:192])
            for q in range(2):
                nc.sync.dma_start(
                    out=out_t[s][:, q * 16 : (q + 1) * 16],
                    in_=out_sb[:, q * 16 : (q + 1) * 16],
                )
    return nc


def _get_nc():
    global _CACHED_NC
    if _CACHED_NC is None:
        nc = _build_nc()
        if not nc.is_finalized():
            nc.finalize()
        _CACHED_NC = nc
    return _CACHED_NC


def _make_in_maps(x, max_, min_, ycbcr_w):
    import ml_dtypes

    bf16 = ml_dtypes.bfloat16
    x16 = np.asarray(x, np.float32).astype(bf16)
    # block-pixel layout: [B, (ci,i,j)=192, blk=(hb,wb)=4096]
    xd = x16.reshape(-1, 3, 64, 8, 64, 8)  # s, ci, hb, i, wb, j
    xd = np.ascontiguousarray(xd.transpose(0, 1, 3, 5, 2, 4))  # s, ci, i, j, hb, wb
    xd = xd.reshape(-1, 192, NBLK)
    ones = np.ones((xd.shape[0], 1, NBLK), bf16)
    xa = np.ascontiguousarray(xd[:, 0:128])
    xb = np.ascontiguousarray(np.concatenate([xd[:, 128:192], ones], axis=1))

    s_aff, b_aff = _affine_coeffs(max_, min_)  # [B, 192] f64 (co,u,v)
    D = _dct_basis()  # [u, i] f64
    y = np.asarray(ycbcr_w, np.float64)  # [co, ci]
    # K[(ci,i,j), (co,u,v)] * s_aff[smp, (co,u,v)]
    kbase = np.einsum("oc,ui,vj->cijouv", y, D, D).reshape(192, 192)
    ks = kbase[None, :, :] * s_aff[:, None, :]  # [B, 192, 192]
    k1 = ks[:, 0:128]
    k2 = np.concatenate([ks[:, 128:192], b_aff[:, None, :]], axis=1)  # [B, 65, 192]

    in_maps = []
    for core in range(NCORES):
        sl = slice(core * BPC, (core + 1) * BPC)
        in_maps.append(
            {
                "xa": xa[sl],
                "xb": xb[sl],
                "k1": k1[sl].astype(bf16),
                "k2": k2[sl].astype(bf16),
            }
        )
    return in_maps


def kernel(x, max_, min_, ycbcr_w, dct_w):
    from concourse.bass_utils import run_bass_kernel_spmd

    nc = _get_nc()
    in_maps = _make_in_maps(x, max_, min_, ycbcr_w)
    res = run_bass_kernel_spmd(nc, in_maps, core_ids=list(range(NCORES)))
    out = np.concatenate([res.results[i]["out"] for i in range(NCORES)], axis=0)
    return _untangle(out)


def _untangle(dev_out):
    """[B, p=128, g=32, 192] device layout -> [B, 192, 64, 64] f32."""
    v = np.asarray(dev_out).astype(np.float32)
    v = v.transpose(0, 2, 1, 3)  # s, g, p, (co,u,v) ; blk = g*128+p = hb*64+wb
    v = v.reshape(-1, 64, 64, 3, 8, 8)  # s, hb, wb, co, u, v
    v = v.transpose(0, 3, 4, 5, 1, 2)  # s, co, u, v, hb, wb
    return np.ascontiguousarray(v.reshape(-1, 192, 64, 64))


# revision 14
# speedup vs baseline: 1.5528x; 1.0697x over previous
"""Trainium2 Bass kernel for nn_DCT: YCbCr 3x3 channel mix + 8x8 block DCT
(stride 8) + repeated min/max normalization collapsed to a per-channel affine.

Sharding: pure data parallel, batch 32 -> 4 samples on each of 8 NeuronCores.

v3 dataflow — direct 2D DCT, everything inside one matmul stage:
  - Host shuffles x to block-pixel layout: xa[s, (ci01,i,j)=128, blk=4096],
    xb[s, (ci2,i,j)+ones=65, blk=4096] (row 64 = 1.0; blk = hb*64+wb).
  - Per-sample rhs constants carry mix, both DCTs, the affine scale AND bias:
      K[(ci,i,j),(co,u,v)] = y[co,ci]*D[u,i]*D[v,j]*s_aff[smp,co,u,v]
      K1 = K rows 0:128, K2 = K rows 128:192 + row 64 = b_aff[smp].
  - Per (s, group of 128 blocks): two accumulating matmuls
      ps[blk128, (co,u,v)] = xa_g^T @ K1 + xb_g^T @ K2   (f32 PSUM)
    give the FINAL normalized DCT output directly.
  - PSUM -> SBUF bf16 copies split between Scalar and Vector engines.
  - One contiguous 1.5MB DMA out per sample; host untangles.
"""

import math
import sys

import numpy as np

for _p in ("/opt/trn_rl_repo", "/opt/pypackages"):
    if _p not in sys.path:
        sys.path.insert(0, _p)

N = 8
IN_CH = 3
EPS = 1e-6
B_FULL = 32
H = 512
W = 512
NCORES = 8
BPC = B_FULL // NCORES  # samples per core
NBLK = 4096  # 64x64 blocks per image
NGRP = NBLK // 128  # 32 groups of 128 blocks

_CACHED_NC = None


def _dct_basis(n=N):
    u = np.arange(n)
    i = np.arange(n)
    b = np.cos(np.pi * np.outer(u, i + 0.5) / n)
    c = np.full(n, math.sqrt(2.0 / n))
    c[0] = math.sqrt(1.0 / n)
    return b * c[:, None]


def _affine_coeffs(max_, min_):
    """Closed form of t -> (t - min)/d applied B_FULL times: out = s*dct + b."""
    m = np.asarray(max_, np.float32)[..., 0, 0]
    n = np.asarray(min_, np.float32)[..., 0, 0]
    d = (m - n + np.float32(EPS)).astype(np.float64)
    r = 1.0 / d
    s = r**B_FULL
    b = -n.astype(np.float64) * (r * (1.0 - s) / (1.0 - r))
    return s, b  # [B, 192] f64


def _build_nc():
    import concourse.mybir as mybir
    import concourse.tile as tile
    from concourse import bacc
    from contextlib import ExitStack

    f32 = mybir.dt.float32
    bf16 = mybir.dt.bfloat16
    nc = bacc.Bacc()
    xa_t = nc.declare_dram_parameter("xa", [BPC, 128, NBLK], bf16, isOutput=False)
    xb_t = nc.declare_dram_parameter("xb", [BPC, 65, NBLK], bf16, isOutput=False)
    k1_t = nc.declare_dram_parameter("k1", [BPC, 128, 192], bf16, isOutput=False)
    k2_t = nc.declare_dram_parameter("k2", [BPC, 65, 192], bf16, isOutput=False)
    # out[s, p=blk%128, g=blk//128, (co,u,v)]
    out_t = nc.declare_dram_parameter("out", [BPC, 128, NGRP, 192], bf16, isOutput=True)

    with ExitStack() as ctx:
        tc = ctx.enter_context(tile.TileContext(nc))
        consts = ctx.enter_context(tc.tile_pool(name="consts", bufs=1))
        xap = ctx.enter_context(tc.tile_pool(name="xap", bufs=3))
        xbp = ctx.enter_context(tc.tile_pool(name="xbp", bufs=3))
        psp = ctx.enter_context(tc.tile_pool(name="psp", bufs=3, space="PSUM"))
        outp = ctx.enter_context(tc.tile_pool(name="outp", bufs=6))

        k1_sb = consts.tile([128, BPC, 192], bf16)
        nc.gpsimd.dma_start(out=k1_sb, in_=k1_t[:].rearrange("s k n -> k s n"))
        k2_sb = consts.tile([65, BPC, 192], bf16)
        nc.gpsimd.dma_start(out=k2_sb, in_=k2_t[:].rearrange("s k n -> k s n"))

        for s in range(BPC):
            xa = xap.tile([128, NBLK], bf16)
            for q in range(4):
                nc.sync.dma_start(
                    out=xa[:, q * 1024 : (q + 1) * 1024],
                    in_=xa_t[s][:, q * 1024 : (q + 1) * 1024],
                )
            xb = xbp.tile([65, NBLK], bf16)
            for q in range(2):
                nc.gpsimd.dma_start(
                    out=xb[:, q * 2048 : (q + 1) * 2048],
                    in_=xb_t[s][:, q * 2048 : (q + 1) * 2048],
                )
            for t4 in range(NGRP // 4):  # 8 PSUM tiles x 4 groups
                ps = psp.tile([128, 4, 256], f32)  # 2 banks; use [:, q, 0:192]
                for q in range(4):
                    g = t4 * 4 + q
                    nc.tensor.matmul(
                        ps[:, q, 0:192],
                        lhsT=xa[:, g * 128 : (g + 1) * 128],
                        rhs=k1_sb[:, s],
                        start=True,
                        stop=False,
                    )
                    nc.tensor.matmul(
                        ps[:, q, 0:192],
                        lhsT=xb[:, g * 128 : (g + 1) * 128],
                        rhs=k2_sb[:, s],
                        start=False,
                        stop=True,
                    )
                dst = outp.tile([128, 4, 192], bf16)
                if t4 % 2 == 0:
                    nc.scalar.copy(out=dst, in_=ps[:, :, 0:192])
                else:
                    nc.vector.tensor_copy(out=dst, in_=ps[:, :, 0:192])
                # out-DMA per copy, issued by the same engine that produced it:
                # keeps the sync queue free for input prefetch (no head-of-line
                # blocking of the next sample's loads behind this sample's out)
                if t4 % 2 == 0:
                    nc.scalar.dma_start(
                        out=out_t[s][:, t4 * 4 : (t4 + 1) * 4], in_=dst
                    )
                else:
                    nc.gpsimd.dma_start(
                        out=out_t[s][:, t4 * 4 : (t4 + 1) * 4], in_=dst
                    )
    return nc


def _get_nc():
    global _CACHED_NC
    if _CACHED_NC is None:
        nc = _build_nc()
        if not nc.is_finalized():
            nc.finalize()
        _CACHED_NC = nc
    return _CACHED_NC


def _make_in_maps(x, max_, min_, ycbcr_w):
    import ml_dtypes

    bf16 = ml_dtypes.bfloat16
    x16 = np.asarray(x, np.float32).astype(bf16)
    # block-pixel layout: [B, (ci,i,j)=192, blk=(hb,wb)=4096]
    xd = x16.reshape(-1, 3, 64, 8, 64, 8)  # s, ci, hb, i, wb, j
    xd = np.ascontiguousarray(xd.transpose(0, 1, 3, 5, 2, 4))  # s, ci, i, j, hb, wb
    xd = xd.reshape(-1, 192, NBLK)
    ones = np.ones((xd.shape[0], 1, NBLK), bf16)
    xa = np.ascontiguousarray(xd[:, 0:128])
    xb = np.ascontiguousarray(np.concatenate([xd[:, 128:192], ones], axis=1))

    s_aff, b_aff = _affine_coeffs(max_, min_)  # [B, 192] f64 (co,u,v)
    D = _dct_basis()  # [u, i] f64
    y = np.asarray(ycbcr_w, np.float64)  # [co, ci]
    # K[(ci,i,j), (co,u,v)] * s_aff[smp, (co,u,v)]
    kbase = np.einsum("oc,ui,vj->cijouv", y, D, D).reshape(192, 192)
    ks = kbase[None, :, :] * s_aff[:, None, :]  # [B, 192, 192]
    k1 = ks[:, 0:128]
    k2 = np.concatenate([ks[:, 128:192], b_aff[:, None, :]], axis=1)  # [B, 65, 192]

    in_maps = []
    for core in range(NCORES):
        sl = slice(core * BPC, (core + 1) * BPC)
        in_maps.append(
            {
                "xa": xa[sl],
                "xb": xb[sl],
                "k1": k1[sl].astype(bf16),
                "k2": k2[sl].astype(bf16),
            }
        )
    return in_maps


def kernel(x, max_, min_, ycbcr_w, dct_w):
    from concourse.bass_utils import run_bass_kernel_spmd

    nc = _get_nc()
    in_maps = _make_in_maps(x, max_, min_, ycbcr_w)
    res = run_bass_kernel_spmd(nc, in_maps, core_ids=list(range(NCORES)))
    out = np.concatenate([res.results[i]["out"] for i in range(NCORES)], axis=0)
    return _untangle(out)


def _untangle(dev_out):
    """[B, p=128, g=32, 192] device layout -> [B, 192, 64, 64] f32."""
    v = np.asarray(dev_out).astype(np.float32)
    v = v.transpose(0, 2, 1, 3)  # s, g, p, (co,u,v) ; blk = g*128+p = hb*64+wb
    v = v.reshape(-1, 64, 64, 3, 8, 8)  # s, hb, wb, co, u, v
    v = v.transpose(0, 3, 4, 5, 1, 2)  # s, co, u, v, hb, wb
    return np.ascontiguousarray(v.reshape(-1, 192, 64, 64))
